# revision 55
# baseline (speedup 1.0000x reference)
"""Multi-head self-attention (B=2, S=2048, D=1024, H=16, causal) on 8 NeuronCores.

Sharding: 32 (batch, head) instances -> 4 heads of one batch per core
(cores 0-3: batch 0, cores 4-7: batch 1; core c owns heads 4*(c%4) .. +3).
Wq/Wk/Wv are split by rows (head dims), Wo by columns; each core computes a
partial y[b] = attn_out_heads @ Wo_cols.T and the host sums the 4 partials
per batch at gather time (tensor-parallel reduce).

Per-core kernel. All matmuls fp16 x fp16 -> fp32 psum. No on-device
transposes:
  QT[256,2048] = wqT.T @ xT        (head-pair tiles: rows 0-63 / 64-127)
  KT likewise; V[2048,256] natural (lhsT = xT chunks), stored per k-tile in
  vabig: per head [V_h (64 cols) | ones (64 cols)].
  Scores computed transposed, blockwise [k-tile 128, q-chunk 512]:
      S^T = KT_h.T @ QT_h   -- two heads row-packed (contraction d=64 at
      partition bases 0 / 64 -> disjoint PE row groups, HW-concurrent).
      Diagonal k-tiles restrict N to the causal range (q >= key base);
      one Exp (scale=1/8) per tile on ScalarE -> P fp16 in SBUF; only the
      [128,128] diagonal block needs the 0/1 tri-mask multiply (DVE).
  AV: lhsT = [V_h | ones64] fp16 [k,128], rhs = P [k,<=512] -> psum
      [128,512] accumulated over k-tiles = unnormalized out^T (rows 0-63)
      + softmax denominators replicated over rows 64-127. Normalize via
      reciprocal_approx_fast(den) + one DVE multiply -> outT fp16.
  y = out_headsT.T @ woT -> [2048, 1024] fp16 partial, DMA'd out in
      [128,1024] blocks (host accumulates partials in fp32).

Startup: junk matmuls warm the PE HAM clock-gate during the input-DMA
window and a dummy exp pulls the ScalarE act-table load off the critical
path. Input DMAs are merged into a handful of multi-dim transfers (each
stripes across all 16 DMA engines) to minimize DGE dispatch time and the
per-semaphore program-teardown cost. The last chunk bridges its
normalization gap with junk matmuls so the PE clock never re-throttles,
runs its den staging + y evacuations on the (then idle) ScalarE, and
normalizes in 128-col pieces so each Wo tile starts as early as possible.
"""
import os
import sys

sys.path.insert(0, "/opt/trn_rl_repo")

import numpy as np

import concourse.bass as bass  # noqa: F401
import concourse.mybir as mybir
from concourse import bacc
from concourse.tile import TileContext
from concourse.bass_utils import run_bass_kernel_spmd

B, S, D = 2, 2048, 1024
H, HD = 16, 64
NCORES = 8
HPC = 4            # heads per core
SC = 512           # q-chunk width
NQC = S // SC      # 4 q-chunks
NKT = S // 128     # 16 k-tiles
F16 = mybir.dt.float16
F32 = mybir.dt.float32
ATTN_SCALE = 1.0 / np.sqrt(HD)

_CACHE = {}


def _build():
    nc = bacc.Bacc("TRN2", target_bir_lowering=False, debug=False, num_devices=NCORES)

    xT_d = nc.declare_dram_parameter("xT", [D, S], F16, isOutput=False)
    wqT_d = nc.declare_dram_parameter("wqT", [D, 256], F16, isOutput=False)
    wkT_d = nc.declare_dram_parameter("wkT", [D, 256], F16, isOutput=False)
    wvT_d = nc.declare_dram_parameter("wvT", [D, 256], F16, isOutput=False)
    woT_d = nc.declare_dram_parameter("woT", [256, D], F16, isOutput=False)
    mask_d = nc.declare_dram_parameter("mask", [128, 256], F16, isOutput=False)
    y_d = nc.declare_dram_parameter("y", [S, D], F16, isOutput=True)

    with TileContext(nc) as tc:
        with (
            tc.tile_pool(name="static", bufs=1) as st,
            tc.tile_pool(name="ppool", bufs=6) as ppool,
            tc.tile_pool(name="rpool", bufs=3) as rpool,
            tc.tile_pool(name="ystage", bufs=3) as ystage,
            tc.tile_pool(name="psA", bufs=2, space="PSUM") as psA,
            tc.tile_pool(name="psS", bufs=2, space="PSUM") as psS,
            tc.tile_pool(name="psV", bufs=1, space="PSUM") as psV,
        ):
            mask = st.tile([128, 256], F16, name="mask", tag="mask")
            wq = st.tile([128, 2048], F16, name="wq", tag="wq")
            wk = st.tile([128, 2048], F16, name="wk", tag="wk")
            wv = st.tile([128, 2048], F16, name="wv", tag="wv")
            wo = st.tile([128, 2048], F16, name="wo", tag="wo")
            xTbig = st.tile([128, 8 * S], F16, name="xTbig", tag="xTbig")
            junk = st.tile([128, 640], F16, name="junk", tag="junk")
            junko = st.tile([128, 8], F16, name="junko", tag="junko")
            # vabig: per k-tile i, head h: cols 512i+128h..+64 = V_h, +64.. = 1
            vabig = st.tile([128, 512 * NKT], F16, name="vabig", tag="vabig")

            def xTc(k, c0, c1):
                return xTbig[:, S * k + c0 : S * k + c1]

            # ---- PE warm-up (HAM un-throttles after ~3.4us of sustained PE
            # activity; keep it busy through the whole input-DMA window so the
            # first projection starts at 2.4 GHz) + act-table preload.
            nc.vector.memset(junk[:], 0.0)
            for w in range(8):
                jacc = psA.tile([128, 512], F32, name="jacc", tag="acc")
                nc.tensor.matmul(
                    jacc[:],
                    junk[:, 0:128],
                    junk[:, 128:640],
                    start=True,
                    stop=True,
                )
            nc.scalar.activation(
                junko[:], junk[:, 0:8], mybir.ActivationFunctionType.Exp, scale=1.0
            )

            # ---- input DMAs. Every dma_start's descriptors stripe over all
            # 16 DMA engines, so one big multi-dim transfer moves at full
            # bandwidth; merging transfers cuts DGE dispatch time AND the
            # per-semaphore teardown cost in the program epilogue.
            # (3+-dim merged transfers fall off the HWDGE fast path into slow
            # software descriptor generation — keep everything plain 2D.)
            for k in range(8):
                nc.gpsimd.dma_start(
                    out=xTc(k, 0, SC), in_=xT_d[128 * k : 128 * k + 128, 0:SC]
                )
            for k in range(8):
                nc.sync.dma_start(
                    out=wq[:, 256 * k : 256 * k + 256],
                    in_=wqT_d[128 * k : 128 * k + 128, :],
                )
            for k in range(8):
                nc.scalar.dma_start(
                    out=wk[:, 256 * k : 256 * k + 256],
                    in_=wkT_d[128 * k : 128 * k + 128, :],
                )
            nc.vector.memset(
                vabig.rearrange("p (i h c) -> p i h c", i=NKT, c=128)[:, :, :, 64:128],
                1.0,
            )
            # input DMA is bandwidth-bound (~6.5MB at ~350GB/s ≈ 19us), so
            # arrival ORDER is what matters: wv then x chunk 1 before x
            # chunks 2-3, else proj(1) — and with it attention(1)'s whole
            # exp stream — waits ~20us on the tail of one big transfer.
            for k in range(8):
                nc.gpsimd.dma_start(
                    out=wv[:, 256 * k : 256 * k + 256],
                    in_=wvT_d[128 * k : 128 * k + 128, :],
                )
            for k in range(4):
                nc.sync.dma_start(
                    out=xTc(k, SC, 2 * SC),
                    in_=xT_d[128 * k : 128 * k + 128, SC : 2 * SC],
                )
            for k in range(4, 8):
                nc.gpsimd.dma_start(
                    out=xTc(k, SC, 2 * SC),
                    in_=xT_d[128 * k : 128 * k + 128, SC : 2 * SC],
                )
            nc.scalar.dma_start(out=mask[:], in_=mask_d[:])
            for cc in range(2):
                nc.scalar.dma_start(
                    out=wo[:, 1024 * cc : 1024 * cc + 1024],
                    in_=woT_d[128 * cc : 128 * cc + 128, :],
                )
            for k in range(4):
                nc.sync.dma_start(
                    out=xTc(k, 2 * SC, S), in_=xT_d[128 * k : 128 * k + 128, 2 * SC : S]
                )
            for k in range(4, 8):
                nc.gpsimd.dma_start(
                    out=xTc(k, 2 * SC, S), in_=xT_d[128 * k : 128 * k + 128, 2 * SC : S]
                )

            QT = [st.tile([128, S], F16, name=f"QT{m}", tag=f"QT{m}") for m in range(2)]
            KT = [st.tile([128, S], F16, name=f"KT{m}", tag=f"KT{m}") for m in range(2)]
            outT = [
                st.tile([128, S], F16, name=f"outT{m}", tag=f"outT{m}")
                for m in range(2)
            ]

            def _proj_chunk(n, dst, w):
                for m in range(2):
                    acc = psA.tile([128, SC], F32, name="acc", tag="acc")
                    for k in range(8):
                        nc.tensor.matmul(
                            acc[:],
                            w[:, 256 * k + 128 * m : 256 * k + 128 * m + 128],
                            xTc(k, SC * n, SC * n + SC),
                            start=(k == 0),
                            stop=(k == 7),
                        )
                    nc.vector.tensor_copy(dst[m][:, SC * n : SC * n + SC], acc[:])

            def proj_q_chunk(n):
                _proj_chunk(n, QT, wq)

            def proj_k_chunk(n):
                _proj_chunk(n, KT, wk)

            def proj_v(i):
                accv = psA.tile([128, 256], F32, name="accv", tag="acc")
                for k in range(8):
                    nc.tensor.matmul(
                        accv[:],
                        xTc(k, 128 * i, 128 * i + 128),
                        wv[:, 256 * k : 256 * k + 256],
                        start=(k == 0),
                        stop=(k == 7),
                    )
                # one strided evacuation into the V columns of vabig
                dst = vabig.rearrange("p (i h c) -> p i h c", i=NKT, c=128)[
                    :, i, :, 0:64
                ]
                src = accv.rearrange("p (h c) -> p h c", c=64)
                nc.vector.tensor_copy(dst, src)

            def attention(jq):
                nkt = 4 * jq + 4  # causal: k-tiles 0 .. 4*jq+3

                def score_exp(hp, kt):
                    t = kt - 4 * jq  # >=0: diagonal k-tile index
                    q0 = 128 * t if t > 0 else 0  # first causal q col
                    sp = psS.tile([128, 1024], F32, name="sp", tag="sp")
                    spv = sp.rearrange("p (u q) -> p u q", q=SC)
                    for u, base in ((0, 0), (1, 64)):
                        nc.tensor.matmul(
                            spv[:, u, q0:SC],
                            KT[hp][base : base + 64, 128 * kt : 128 * kt + 128],
                            QT[hp][base : base + 64, SC * jq + q0 : SC * jq + SC],
                            start=True,
                            stop=True,
                        )
                    pt = ppool.tile([128, 1024], F16, name="pt", tag="pt")
                    ptv = pt.rearrange("p (u q) -> p u q", q=SC)
                    nc.scalar.activation(
                        ptv[:, :, q0:SC],
                        spv[:, :, q0:SC],
                        mybir.ActivationFunctionType.Exp,
                        scale=float(ATTN_SCALE),
                    )
                    if t >= 0:  # diagonal k-tile: tri-mask its 128 q cols
                        nc.vector.tensor_mul(
                            ptv[:, :, q0 : q0 + 128],
                            ptv[:, :, q0 : q0 + 128],
                            mask.rearrange("p (u q) -> p u q", q=128),
                        )
                    return pt, ptv, q0

                pre = None  # pre-emitted (pt, ptv, q0) for hp1's kt=0
                for hp in range(2):
                    av = [
                        psV.tile([128, SC], F32, name=f"av{u}", tag=f"av{u}")
                        for u in range(2)
                    ]
                    last_pt = None
                    if hp == 0:
                        # chunk opener: the first 4 k-tiles' score/exp stream
                        # needs only Q and OLD K columns, so emit it BEFORE
                        # this chunk's K and V projections — the exp stream
                        # restarts ~6us earlier at every chunk boundary and
                        # the diagonal scores / AV matmuls catch up after.
                        # (for jq=0 the first 4 k-tiles ARE the diagonal and
                        # read K(0), so K(0) is projected before attention(0))
                        ses = [score_exp(0, kt) for kt in range(4)]
                        if jq > 0:
                            proj_k_chunk(jq)
                        for i in range(4 * jq, 4 * jq + 4):
                            proj_v(i)
                        for kt in range(nkt):
                            if kt < 4:
                                pt, ptv, q0 = ses[kt]
                            else:
                                pt, ptv, q0 = score_exp(0, kt)
                            last_pt = pt
                            for u in range(2):
                                nc.tensor.matmul(
                                    av[u][:, q0:SC],
                                    vabig[
                                        :, 512 * kt + 128 * u : 512 * kt + 128 * u + 128
                                    ],
                                    ptv[:, u, q0:SC],
                                    start=(kt == 0),
                                    stop=(kt == nkt - 1),
                                )
                    else:
                        for kt in range(nkt):
                            if kt == 0 and pre is not None:
                                pt, ptv, q0 = pre
                            else:
                                pt, ptv, q0 = score_exp(hp, kt)
                            last_pt = pt
                            for u in range(2):
                                h = 2 * hp + u
                                nc.tensor.matmul(
                                    av[u][:, q0:SC],
                                    vabig[
                                        :, 512 * kt + 128 * h : 512 * kt + 128 * h + 128
                                    ],
                                    ptv[:, u, q0:SC],
                                    start=(kt == 0),
                                    stop=(kt == nkt - 1),
                                )
                    if hp == 0:
                        # pre-emit hp1's first score pair + exp so the exp
                        # stream isn't head-of-line-blocked behind fill work
                        # across the head-pair boundary (observed 3-4us
                        # ScalarE stalls otherwise).
                        pre = score_exp(1, 0)
                    # normalize: rows 64-127 of av hold the softmax denom
                    # replicated 64x. reciprocal_approx_fast mis-reads PSUM
                    # operands (verified on hw): den staged through SBUF.
                    last = jq == NQC - 1 and hp == 1
                    if last:
                        # tail: pipeline the normalization at 128-col piece
                        # granularity across ScalarE (den u0) and VectorE
                        # (everything else) so the first wo i-tile starts
                        # ~1us earlier than with whole-row staging.
                        dens, recs = [], []
                        for u in range(2):
                            den = rpool.tile([64, SC], F32, name="den", tag="den")
                            if u == 0:
                                nc.scalar.activation(
                                    den[:], av[u][64:128, :],
                                    mybir.ActivationFunctionType.Copy, scale=1.0,
                                )
                            else:
                                nc.vector.tensor_copy(den[:], av[u][64:128, :])
                            rec = rpool.tile([64, SC], F32, name="rec", tag="rec")
                            dens.append(den)
                            recs.append(rec)
                        # keep the PE clock warm across the normalization gap.
                        # rhs reads the final P tile so these only become
                        # ready once the tail begins (else the scheduler
                        # hoists them into earlier idle slots).
                        for w in range(6):
                            jacc = psA.tile([128, 512], F32, name="jacc", tag="acc")
                            nc.tensor.matmul(
                                jacc[:],
                                junk[:, 0:128],
                                last_pt[:, 256:768],
                                start=True,
                                stop=True,
                            )
                        for piece in range(4):
                            cl = slice(128 * piece, 128 * piece + 128)
                            gl = slice(
                                SC * jq + 128 * piece, SC * jq + 128 * piece + 128
                            )
                            for u in range(2):
                                nc.vector.reciprocal_approx_fast(
                                    recs[u][:, cl], dens[u][:, cl]
                                )
                            for u in range(2):
                                nc.vector.tensor_mul(
                                    outT[hp][64 * u : 64 * u + 64, gl],
                                    av[u][0:64, cl],
                                    recs[u][:, cl],
                                )
                    else:
                        for u in range(2):
                            den = rpool.tile([64, SC], F32, name="den", tag="den")
                            nc.vector.tensor_copy(den[:], av[u][64:128, :])
                            rec = rpool.tile([64, SC], F32, name="rec", tag="rec")
                            nc.vector.reciprocal_approx_fast(rec[:], den[:])
                            nc.vector.tensor_mul(
                                outT[hp][64 * u : 64 * u + 64, SC * jq : SC * jq + SC],
                                av[u][0:64, :],
                                rec[:],
                            )

            def wo_chunk(jq):
                last = jq == NQC - 1
                for i in range(4 * jq, 4 * jq + 4):
                    ys = ystage.tile([128, 1024], F16, name="ys", tag="ys")
                    for n in range(2):
                        if last and i >= 4 * jq + 2:
                            # attention is over; borrow the (now idle) score
                            # psum banks to double the yp slots in flight.
                            yp = psS.tile([128, 1024], F32, name="ypl", tag="sp")[
                                :, 0:512
                            ]
                        else:
                            yp = psA.tile([128, 512], F32, name="yp", tag="acc")
                        for cc in range(2):
                            nc.tensor.matmul(
                                yp[:],
                                outT[cc][:, 128 * i : 128 * i + 128],
                                wo[:, 1024 * cc + 512 * n : 1024 * cc + 512 * n + 512],
                                start=(cc == 0),
                                stop=(cc == 1),
                            )
                        ysl = ys[:, 512 * n : 512 * n + 512]
                        if last and n == 1:
                            # ACT is idle after the final exp; split the two
                            # evacuations across ScalarE and VectorE.
                            nc.scalar.activation(
                                ysl, yp[:], mybir.ActivationFunctionType.Copy,
                                scale=1.0,
                            )
                        else:
                            nc.vector.tensor_copy(ysl, yp[:])
                    eng = nc.sync if i % 2 == 0 else nc.gpsimd
                    eng.dma_start(out=y_d[128 * i : 128 * i + 128, :], in_=ys[:])

            proj_q_chunk(0)
            proj_k_chunk(0)
            for n in range(NQC):
                attention(n)
                if n + 1 < NQC:
                    proj_q_chunk(n + 1)
                wo_chunk(n)

    nc.compile()
    return nc


def _mask_np():
    # [128, 256]: tri pattern duplicated for the two row-packed heads.
    r = np.arange(128)[:, None]
    c = np.arange(128)[None, :]
    tri = (r <= c).astype(np.float16)
    return np.concatenate([tri, tri], axis=1)


def kernel(x, Wq, Wk, Wv, Wo):
    x = np.asarray(x, dtype=np.float32)
    Wq = np.asarray(Wq, dtype=np.float32)
    Wk = np.asarray(Wk, dtype=np.float32)
    Wv = np.asarray(Wv, dtype=np.float32)
    Wo = np.asarray(Wo, dtype=np.float32)

    if "nc" not in _CACHE:
        _CACHE["nc"] = _build()
    nc = _CACHE["nc"]

    masks = _mask_np()
    xT = [np.ascontiguousarray(x[b].T).astype(np.float16) for b in range(B)]
    in_maps = []
    for c in range(NCORES):
        b, g = c // 4, c % 4
        rows = slice(256 * g, 256 * g + 256)
        in_maps.append(
            {
                "xT": xT[b],
                "wqT": np.ascontiguousarray(Wq[rows].T).astype(np.float16),
                "wkT": np.ascontiguousarray(Wk[rows].T).astype(np.float16),
                "wvT": np.ascontiguousarray(Wv[rows].T).astype(np.float16),
                "woT": np.ascontiguousarray(Wo[:, rows].T).astype(np.float16),
                "mask": masks,
            }
        )

    trace = False
    trace_tmpdir = None
    if os.environ.get("KERNEL_TRACE") == "1":
        try:
            from trn_agent_boot.trn_boot import _ntff_profile_via_ctypes

            try:
                from antenv.axon_hooks import (
                    get_axon_ntff_profile_hook,
                    set_axon_ntff_profile_hook,
                )
            except ImportError:
                # This image's antenv lacks the hook-registry module; provide
                # an equivalent in-process registry so bass_utils can find it.
                import types

                import antenv

                _mod = types.ModuleType("antenv.axon_hooks")
                _mod._hook = None

                def get_axon_ntff_profile_hook(_m=_mod):
                    return _m._hook

                def set_axon_ntff_profile_hook(h, _m=_mod):
                    _m._hook = h

                _mod.get_axon_ntff_profile_hook = get_axon_ntff_profile_hook
                _mod.set_axon_ntff_profile_hook = set_axon_ntff_profile_hook
                sys.modules["antenv.axon_hooks"] = _mod
                antenv.axon_hooks = _mod

            if get_axon_ntff_profile_hook() is None:
                set_axon_ntff_profile_hook(
                    _ntff_profile_via_ctypes("/opt/axon/libaxon_pjrt.so")
                )
            trace = get_axon_ntff_profile_hook() is not None
            if trace:
                import shutil

                trace_tmpdir = "/tmp/ktrace"
                shutil.rmtree(trace_tmpdir, ignore_errors=True)
                os.makedirs(trace_tmpdir, exist_ok=True)
        except Exception:
            trace = False
            trace_tmpdir = None

    res = run_bass_kernel_spmd(
        nc, in_maps, core_ids=list(range(NCORES)), trace=trace, tmpdir=trace_tmpdir
    )
    _CACHE["exec_time_ns"] = res.exec_time_ns
    _CACHE["res"] = res
    y = np.zeros((B, S, D), dtype=np.float32)
    for c in range(NCORES):
        y[c // 4] += res.results[c]["y"].astype(np.float32)
    return y


# revision 56
# speedup vs baseline: 1.0365x; 1.0365x over previous
"""Multi-head self-attention (B=2, S=2048, D=1024, H=16, causal) on 8 NeuronCores.

Sharding: 32 (batch, head) instances -> 4 heads of one batch per core
(cores 0-3: batch 0, cores 4-7: batch 1; core c owns heads 4*(c%4) .. +3).
Wq/Wk/Wv are split by rows (head dims), Wo by columns; each core computes a
partial y[b] = attn_out_heads @ Wo_cols.T and the host sums the 4 partials
per batch at gather time (tensor-parallel reduce).

Per-core kernel. All matmuls fp16 x fp16 -> fp32 psum. No on-device
transposes:
  QT[256,2048] = wqT.T @ xT        (head-pair tiles: rows 0-63 / 64-127)
  KT likewise; V[2048,256] natural (lhsT = xT chunks), stored per k-tile in
  vabig: per head [V_h (64 cols) | ones (64 cols)].
  Scores computed transposed, blockwise [k-tile 128, q-chunk 512]:
      S^T = KT_h.T @ QT_h   -- two heads row-packed (contraction d=64 at
      partition bases 0 / 64 -> disjoint PE row groups, HW-concurrent).
      Diagonal k-tiles restrict N to the causal range (q >= key base);
      one Exp (scale=1/8) per tile on ScalarE -> P fp16 in SBUF; only the
      [128,128] diagonal block needs the 0/1 tri-mask multiply (DVE).
  AV: lhsT = [V_h | ones64] fp16 [k,128], rhs = P [k,<=512] -> psum
      [128,512] accumulated over k-tiles = unnormalized out^T (rows 0-63)
      + softmax denominators replicated over rows 64-127. Normalize via
      reciprocal_approx_fast(den) + one DVE multiply -> outT fp16.
  y = out_headsT.T @ woT -> [2048, 1024] fp16 partial, DMA'd out in
      [128,1024] blocks (host accumulates partials in fp32).

Startup: junk matmuls warm the PE HAM clock-gate during the input-DMA
window and a dummy exp pulls the ScalarE act-table load off the critical
path. Input DMAs are merged into a handful of multi-dim transfers (each
stripes across all 16 DMA engines) to minimize DGE dispatch time and the
per-semaphore program-teardown cost. The last chunk bridges its
normalization gap with junk matmuls so the PE clock never re-throttles,
runs its den staging + y evacuations on the (then idle) ScalarE, and
normalizes in 128-col pieces so each Wo tile starts as early as possible.
"""
import os
import sys

sys.path.insert(0, "/opt/trn_rl_repo")

import numpy as np

import concourse.bass as bass  # noqa: F401
import concourse.mybir as mybir
from concourse import bacc
from concourse.tile import TileContext
from concourse.bass_utils import run_bass_kernel_spmd

B, S, D = 2, 2048, 1024
H, HD = 16, 64
NCORES = 8
HPC = 4            # heads per core
SC = 512           # q-chunk width
NQC = S // SC      # 4 q-chunks
NKT = S // 128     # 16 k-tiles
F16 = mybir.dt.float16
F32 = mybir.dt.float32
ATTN_SCALE = 1.0 / np.sqrt(HD)

_CACHE = {}


def _build():
    nc = bacc.Bacc("TRN2", target_bir_lowering=False, debug=False, num_devices=NCORES)

    xT_d = nc.declare_dram_parameter("xT", [D, S], F16, isOutput=False)
    wqT_d = nc.declare_dram_parameter("wqT", [D, 256], F16, isOutput=False)
    wkT_d = nc.declare_dram_parameter("wkT", [D, 256], F16, isOutput=False)
    wvT_d = nc.declare_dram_parameter("wvT", [D, 256], F16, isOutput=False)
    woT_d = nc.declare_dram_parameter("woT", [256, D], F16, isOutput=False)
    mask_d = nc.declare_dram_parameter("mask", [128, 256], F16, isOutput=False)
    y_d = nc.declare_dram_parameter("y", [S, D], F16, isOutput=True)

    with TileContext(nc) as tc:
        with (
            tc.tile_pool(name="static", bufs=1) as st,
            tc.tile_pool(name="ppool", bufs=6) as ppool,
            tc.tile_pool(name="rpool", bufs=3) as rpool,
            tc.tile_pool(name="ystage", bufs=3) as ystage,
            tc.tile_pool(name="psA", bufs=2, space="PSUM") as psA,
            tc.tile_pool(name="psS", bufs=2, space="PSUM") as psS,
            tc.tile_pool(name="psV", bufs=1, space="PSUM") as psV,
        ):
            mask = st.tile([128, 256], F16, name="mask", tag="mask")
            wq = st.tile([128, 2048], F16, name="wq", tag="wq")
            wk = st.tile([128, 2048], F16, name="wk", tag="wk")
            wv = st.tile([128, 2048], F16, name="wv", tag="wv")
            wo = st.tile([128, 2048], F16, name="wo", tag="wo")
            xTbig = st.tile([128, 8 * S], F16, name="xTbig", tag="xTbig")
            junk = st.tile([128, 640], F16, name="junk", tag="junk")
            junko = st.tile([128, 8], F16, name="junko", tag="junko")
            # vabig: per k-tile i, head h: cols 512i+128h..+64 = V_h, +64.. = 1
            vabig = st.tile([128, 512 * NKT], F16, name="vabig", tag="vabig")

            def xTc(k, c0, c1):
                return xTbig[:, S * k + c0 : S * k + c1]

            # ---- PE warm-up (HAM un-throttles after ~3.4us of sustained PE
            # activity; keep it busy through the whole input-DMA window so the
            # first projection starts at 2.4 GHz) + act-table preload.
            nc.vector.memset(junk[:], 0.0)
            for w in range(8):
                jacc = psA.tile([128, 512], F32, name="jacc", tag="acc")
                nc.tensor.matmul(
                    jacc[:],
                    junk[:, 0:128],
                    junk[:, 128:640],
                    start=True,
                    stop=True,
                )
            nc.scalar.activation(
                junko[:], junk[:, 0:8], mybir.ActivationFunctionType.Exp, scale=1.0
            )

            # ---- input DMAs. Every dma_start's descriptors stripe over all
            # 16 DMA engines, so one big multi-dim transfer moves at full
            # bandwidth; merging transfers cuts DGE dispatch time AND the
            # per-semaphore teardown cost in the program epilogue.
            # (3+-dim merged transfers fall off the HWDGE fast path into slow
            # software descriptor generation — keep everything plain 2D.)
            for k in range(8):
                nc.gpsimd.dma_start(
                    out=xTc(k, 0, SC), in_=xT_d[128 * k : 128 * k + 128, 0:SC]
                )
            for k in range(8):
                nc.sync.dma_start(
                    out=wq[:, 256 * k : 256 * k + 256],
                    in_=wqT_d[128 * k : 128 * k + 128, :],
                )
            for k in range(8):
                nc.scalar.dma_start(
                    out=wk[:, 256 * k : 256 * k + 256],
                    in_=wkT_d[128 * k : 128 * k + 128, :],
                )
            nc.vector.memset(
                vabig.rearrange("p (i h c) -> p i h c", i=NKT, c=128)[:, :, :, 64:128],
                1.0,
            )
            # input DMA is bandwidth-bound (~6.5MB at ~350GB/s ≈ 19us), so
            # arrival ORDER is what matters: wv then x chunk 1 before x
            # chunks 2-3, else proj(1) — and with it attention(1)'s whole
            # exp stream — waits ~20us on the tail of one big transfer.
            for k in range(8):
                nc.gpsimd.dma_start(
                    out=wv[:, 256 * k : 256 * k + 256],
                    in_=wvT_d[128 * k : 128 * k + 128, :],
                )
            for k in range(4):
                nc.sync.dma_start(
                    out=xTc(k, SC, 2 * SC),
                    in_=xT_d[128 * k : 128 * k + 128, SC : 2 * SC],
                )
            for k in range(4, 8):
                nc.gpsimd.dma_start(
                    out=xTc(k, SC, 2 * SC),
                    in_=xT_d[128 * k : 128 * k + 128, SC : 2 * SC],
                )
            nc.scalar.dma_start(out=mask[:], in_=mask_d[:])
            for cc in range(2):
                nc.scalar.dma_start(
                    out=wo[:, 1024 * cc : 1024 * cc + 1024],
                    in_=woT_d[128 * cc : 128 * cc + 128, :],
                )
            for k in range(4):
                nc.sync.dma_start(
                    out=xTc(k, 2 * SC, S), in_=xT_d[128 * k : 128 * k + 128, 2 * SC : S]
                )
            for k in range(4, 8):
                nc.gpsimd.dma_start(
                    out=xTc(k, 2 * SC, S), in_=xT_d[128 * k : 128 * k + 128, 2 * SC : S]
                )

            QT = [st.tile([128, S], F16, name=f"QT{m}", tag=f"QT{m}") for m in range(2)]
            KT = [st.tile([128, S], F16, name=f"KT{m}", tag=f"KT{m}") for m in range(2)]
            outT = [
                st.tile([128, S], F16, name=f"outT{m}", tag=f"outT{m}")
                for m in range(2)
            ]

            def proj_qk_chunk(n):
                for dst, w in ((QT, wq), (KT, wk)):
                    for m in range(2):
                        acc = psA.tile([128, SC], F32, name="acc", tag="acc")
                        for k in range(8):
                            nc.tensor.matmul(
                                acc[:],
                                w[:, 256 * k + 128 * m : 256 * k + 128 * m + 128],
                                xTc(k, SC * n, SC * n + SC),
                                start=(k == 0),
                                stop=(k == 7),
                            )
                        nc.vector.tensor_copy(dst[m][:, SC * n : SC * n + SC], acc[:])

            def proj_v(i):
                accv = psA.tile([128, 256], F32, name="accv", tag="acc")
                for k in range(8):
                    nc.tensor.matmul(
                        accv[:],
                        xTc(k, 128 * i, 128 * i + 128),
                        wv[:, 256 * k : 256 * k + 256],
                        start=(k == 0),
                        stop=(k == 7),
                    )
                # one strided evacuation into the V columns of vabig
                dst = vabig.rearrange("p (i h c) -> p i h c", i=NKT, c=128)[
                    :, i, :, 0:64
                ]
                src = accv.rearrange("p (h c) -> p h c", c=64)
                nc.vector.tensor_copy(dst, src)

            def attention(jq):
                nkt = 4 * jq + 4  # causal: k-tiles 0 .. 4*jq+3

                def score_exp(hp, kt):
                    t = kt - 4 * jq  # >=0: diagonal k-tile index
                    q0 = 128 * t if t > 0 else 0  # first causal q col
                    sp = psS.tile([128, 1024], F32, name="sp", tag="sp")
                    spv = sp.rearrange("p (u q) -> p u q", q=SC)
                    for u, base in ((0, 0), (1, 64)):
                        nc.tensor.matmul(
                            spv[:, u, q0:SC],
                            KT[hp][base : base + 64, 128 * kt : 128 * kt + 128],
                            QT[hp][base : base + 64, SC * jq + q0 : SC * jq + SC],
                            start=True,
                            stop=True,
                        )
                    pt = ppool.tile([128, 1024], F16, name="pt", tag="pt")
                    ptv = pt.rearrange("p (u q) -> p u q", q=SC)
                    nc.scalar.activation(
                        ptv[:, :, q0:SC],
                        spv[:, :, q0:SC],
                        mybir.ActivationFunctionType.Exp,
                        scale=float(ATTN_SCALE),
                    )
                    if t >= 0:  # diagonal k-tile: tri-mask its 128 q cols
                        nc.vector.tensor_mul(
                            ptv[:, :, q0 : q0 + 128],
                            ptv[:, :, q0 : q0 + 128],
                            mask.rearrange("p (u q) -> p u q", q=128),
                        )
                    return pt, ptv, q0

                pre = None  # pre-emitted (pt, ptv, q0) for hp1's kt=0
                for hp in range(2):
                    av = [
                        psV.tile([128, SC], F32, name=f"av{u}", tag=f"av{u}")
                        for u in range(2)
                    ]
                    last_pt = None
                    if jq == 0 and hp == 0:
                        # chunk 0 opener: the score/exp stream needs only
                        # Q/K, so emit all four k-tiles' scores+exps BEFORE
                        # the V projections (whose wv input is still in
                        # flight); the AV matmuls catch up afterwards.
                        ses = [score_exp(0, kt) for kt in range(nkt)]
                        for i in range(4):
                            proj_v(i)
                        for kt in range(nkt):
                            pt, ptv, q0 = ses[kt]
                            last_pt = pt
                            for u in range(2):
                                nc.tensor.matmul(
                                    av[u][:, q0:SC],
                                    vabig[
                                        :, 512 * kt + 128 * u : 512 * kt + 128 * u + 128
                                    ],
                                    ptv[:, u, q0:SC],
                                    start=(kt == 0),
                                    stop=(kt == nkt - 1),
                                )
                    else:
                        for kt in range(nkt):
                            if hp == 1 and kt == 0 and pre is not None:
                                pt, ptv, q0 = pre
                            else:
                                pt, ptv, q0 = score_exp(hp, kt)
                            last_pt = pt
                            for u in range(2):
                                h = 2 * hp + u
                                nc.tensor.matmul(
                                    av[u][:, q0:SC],
                                    vabig[
                                        :, 512 * kt + 128 * h : 512 * kt + 128 * h + 128
                                    ],
                                    ptv[:, u, q0:SC],
                                    start=(kt == 0),
                                    stop=(kt == nkt - 1),
                                )
                    if hp == 0:
                        # pre-emit hp1's first score pair + exp so the exp
                        # stream isn't head-of-line-blocked behind fill work
                        # across the head-pair boundary (observed 3-4us
                        # ScalarE stalls otherwise).
                        pre = score_exp(1, 0)
                    # normalize: rows 64-127 of av hold the softmax denom
                    # replicated 64x. reciprocal_approx_fast mis-reads PSUM
                    # operands (verified on hw): den staged through SBUF.
                    last = jq == NQC - 1 and hp == 1
                    if last:
                        # tail: pipeline the normalization at 128-col piece
                        # granularity across ScalarE (den u0) and VectorE
                        # (everything else) so the first wo i-tile starts
                        # ~1us earlier than with whole-row staging.
                        dens, recs = [], []
                        for u in range(2):
                            den = rpool.tile([64, SC], F32, name="den", tag="den")
                            if u == 0:
                                nc.scalar.activation(
                                    den[:], av[u][64:128, :],
                                    mybir.ActivationFunctionType.Copy, scale=1.0,
                                )
                            else:
                                nc.vector.tensor_copy(den[:], av[u][64:128, :])
                            rec = rpool.tile([64, SC], F32, name="rec", tag="rec")
                            dens.append(den)
                            recs.append(rec)
                        # keep the PE clock warm across the normalization gap.
                        # rhs reads the final P tile so these only become
                        # ready once the tail begins (else the scheduler
                        # hoists them into earlier idle slots).
                        for w in range(6):
                            jacc = psA.tile([128, 512], F32, name="jacc", tag="acc")
                            nc.tensor.matmul(
                                jacc[:],
                                junk[:, 0:128],
                                last_pt[:, 256:768],
                                start=True,
                                stop=True,
                            )
                        for piece in range(4):
                            cl = slice(128 * piece, 128 * piece + 128)
                            gl = slice(
                                SC * jq + 128 * piece, SC * jq + 128 * piece + 128
                            )
                            for u in range(2):
                                nc.vector.reciprocal_approx_fast(
                                    recs[u][:, cl], dens[u][:, cl]
                                )
                            for u in range(2):
                                nc.vector.tensor_mul(
                                    outT[hp][64 * u : 64 * u + 64, gl],
                                    av[u][0:64, cl],
                                    recs[u][:, cl],
                                )
                    else:
                        for u in range(2):
                            den = rpool.tile([64, SC], F32, name="den", tag="den")
                            nc.vector.tensor_copy(den[:], av[u][64:128, :])
                            rec = rpool.tile([64, SC], F32, name="rec", tag="rec")
                            nc.vector.reciprocal_approx_fast(rec[:], den[:])
                            nc.vector.tensor_mul(
                                outT[hp][64 * u : 64 * u + 64, SC * jq : SC * jq + SC],
                                av[u][0:64, :],
                                rec[:],
                            )

            def wo_chunk(jq):
                last = jq == NQC - 1
                for i in range(4 * jq, 4 * jq + 4):
                    ys = ystage.tile([128, 1024], F16, name="ys", tag="ys")
                    for n in range(2):
                        if last and i >= 4 * jq + 2:
                            # attention is over; borrow the (now idle) score
                            # psum banks to double the yp slots in flight.
                            yp = psS.tile([128, 1024], F32, name="ypl", tag="sp")[
                                :, 0:512
                            ]
                        else:
                            yp = psA.tile([128, 512], F32, name="yp", tag="acc")
                        for cc in range(2):
                            nc.tensor.matmul(
                                yp[:],
                                outT[cc][:, 128 * i : 128 * i + 128],
                                wo[:, 1024 * cc + 512 * n : 1024 * cc + 512 * n + 512],
                                start=(cc == 0),
                                stop=(cc == 1),
                            )
                        ysl = ys[:, 512 * n : 512 * n + 512]
                        if last and n == 1:
                            # ACT is idle after the final exp; split the two
                            # evacuations across ScalarE and VectorE.
                            nc.scalar.activation(
                                ysl, yp[:], mybir.ActivationFunctionType.Copy,
                                scale=1.0,
                            )
                        else:
                            nc.vector.tensor_copy(ysl, yp[:])
                    eng = nc.sync if i % 2 == 0 else nc.gpsimd
                    eng.dma_start(out=y_d[128 * i : 128 * i + 128, :], in_=ys[:])

            proj_qk_chunk(0)
            for n in range(NQC):
                attention(n)
                if n + 1 < NQC:
                    proj_qk_chunk(n + 1)
                    for i in range(4 * (n + 1), 4 * (n + 1) + 4):
                        proj_v(i)
                wo_chunk(n)

    nc.compile()
    return nc


def _mask_np():
    # [128, 256]: tri pattern duplicated for the two row-packed heads.
    r = np.arange(128)[:, None]
    c = np.arange(128)[None, :]
    tri = (r <= c).astype(np.float16)
    return np.concatenate([tri, tri], axis=1)


def kernel(x, Wq, Wk, Wv, Wo):
    x = np.asarray(x, dtype=np.float32)
    Wq = np.asarray(Wq, dtype=np.float32)
    Wk = np.asarray(Wk, dtype=np.float32)
    Wv = np.asarray(Wv, dtype=np.float32)
    Wo = np.asarray(Wo, dtype=np.float32)

    if "nc" not in _CACHE:
        _CACHE["nc"] = _build()
    nc = _CACHE["nc"]

    masks = _mask_np()
    xT = [np.ascontiguousarray(x[b].T).astype(np.float16) for b in range(B)]
    in_maps = []
    for c in range(NCORES):
        b, g = c // 4, c % 4
        rows = slice(256 * g, 256 * g + 256)
        in_maps.append(
            {
                "xT": xT[b],
                "wqT": np.ascontiguousarray(Wq[rows].T).astype(np.float16),
                "wkT": np.ascontiguousarray(Wk[rows].T).astype(np.float16),
                "wvT": np.ascontiguousarray(Wv[rows].T).astype(np.float16),
                "woT": np.ascontiguousarray(Wo[:, rows].T).astype(np.float16),
                "mask": masks,
            }
        )

    trace = False
    trace_tmpdir = None
    if os.environ.get("KERNEL_TRACE") == "1":
        try:
            from trn_agent_boot.trn_boot import _ntff_profile_via_ctypes

            try:
                from antenv.axon_hooks import (
                    get_axon_ntff_profile_hook,
                    set_axon_ntff_profile_hook,
                )
            except ImportError:
                # This image's antenv lacks the hook-registry module; provide
                # an equivalent in-process registry so bass_utils can find it.
                import types

                import antenv

                _mod = types.ModuleType("antenv.axon_hooks")
                _mod._hook = None

                def get_axon_ntff_profile_hook(_m=_mod):
                    return _m._hook

                def set_axon_ntff_profile_hook(h, _m=_mod):
                    _m._hook = h

                _mod.get_axon_ntff_profile_hook = get_axon_ntff_profile_hook
                _mod.set_axon_ntff_profile_hook = set_axon_ntff_profile_hook
                sys.modules["antenv.axon_hooks"] = _mod
                antenv.axon_hooks = _mod

            if get_axon_ntff_profile_hook() is None:
                set_axon_ntff_profile_hook(
                    _ntff_profile_via_ctypes("/opt/axon/libaxon_pjrt.so")
                )
            trace = get_axon_ntff_profile_hook() is not None
            if trace:
                import shutil

                trace_tmpdir = "/tmp/ktrace"
                shutil.rmtree(trace_tmpdir, ignore_errors=True)
                os.makedirs(trace_tmpdir, exist_ok=True)
        except Exception:
            trace = False
            trace_tmpdir = None

    res = run_bass_kernel_spmd(
        nc, in_maps, core_ids=list(range(NCORES)), trace=trace, tmpdir=trace_tmpdir
    )
    _CACHE["exec_time_ns"] = res.exec_time_ns
    _CACHE["res"] = res
    y = np.zeros((B, S, D), dtype=np.float32)
    for c in range(NCORES):
        y[c // 4] += res.results[c]["y"].astype(np.float32)
    return y


# revision 61
# speedup vs baseline: 1.0501x; 1.0131x over previous
"""Multi-head self-attention (B=2, S=2048, D=1024, H=16, causal) on 8 NeuronCores.

Sharding: 32 (batch, head) instances -> 4 heads of one batch per core
(cores 0-3: batch 0, cores 4-7: batch 1; core c owns heads 4*(c%4) .. +3).
Wq/Wk/Wv are split by rows (head dims), Wo by columns; each core computes a
partial y[b] = attn_out_heads @ Wo_cols.T and the host sums the 4 partials
per batch at gather time (tensor-parallel reduce).

Per-core kernel. All matmuls fp16 x fp16 -> fp32 psum. No on-device
transposes:
  QT[256,2048] = wqT.T @ xT        (head-pair tiles: rows 0-63 / 64-127)
  KT likewise; V[2048,256] natural (lhsT = xT chunks), stored per k-tile in
  vabig: per head [V_h (64 cols) | ones (64 cols)].
  Scores computed transposed, blockwise [k-tile 128, q-chunk 512]:
      S^T = KT_h.T @ QT_h   -- two heads row-packed (contraction d=64 at
      partition bases 0 / 64 -> disjoint PE row groups, HW-concurrent).
      Diagonal k-tiles restrict N to the causal range (q >= key base);
      one Exp (scale=1/8) per tile on ScalarE -> P fp16 in SBUF; only the
      [128,128] diagonal block needs the 0/1 tri-mask multiply (DVE).
  AV: lhsT = [V_h | ones64] fp16 [k,128], rhs = P [k,<=512] -> psum
      [128,512] accumulated over k-tiles = unnormalized out^T (rows 0-63)
      + softmax denominators replicated over rows 64-127. Normalize via
      reciprocal_approx_fast(den) + one DVE multiply -> outT fp16.
  y = out_headsT.T @ woT -> [2048, 1024] fp16 partial, DMA'd out in
      [128,1024] blocks (host accumulates partials in fp32).

Startup: junk matmuls warm the PE HAM clock-gate during the input-DMA
window and a dummy exp pulls the ScalarE act-table load off the critical
path. Input DMAs are merged into a handful of multi-dim transfers (each
stripes across all 16 DMA engines) to minimize DGE dispatch time and the
per-semaphore program-teardown cost. The last chunk bridges its
normalization gap with junk matmuls so the PE clock never re-throttles,
runs its den staging + y evacuations on the (then idle) ScalarE, and
normalizes in 128-col pieces so each Wo tile starts as early as possible.
"""
import os
import sys

sys.path.insert(0, "/opt/trn_rl_repo")

import numpy as np

import concourse.bass as bass  # noqa: F401
import concourse.mybir as mybir
from concourse import bacc
from concourse.tile import TileContext
from concourse.bass_utils import run_bass_kernel_spmd

B, S, D = 2, 2048, 1024
H, HD = 16, 64
NCORES = 8
HPC = 4            # heads per core
SC = 512           # q-chunk width
NQC = S // SC      # 4 q-chunks
NKT = S // 128     # 16 k-tiles
F16 = mybir.dt.float16
F32 = mybir.dt.float32
ATTN_SCALE = 1.0 / np.sqrt(HD)

_CACHE = {}


def _build():
    nc = bacc.Bacc("TRN2", target_bir_lowering=False, debug=False, num_devices=NCORES)

    xT_d = nc.declare_dram_parameter("xT", [D, S], F16, isOutput=False)
    wqT_d = nc.declare_dram_parameter("wqT", [D, 256], F16, isOutput=False)
    wkT_d = nc.declare_dram_parameter("wkT", [D, 256], F16, isOutput=False)
    wvT_d = nc.declare_dram_parameter("wvT", [D, 256], F16, isOutput=False)
    woT_d = nc.declare_dram_parameter("woT", [256, D], F16, isOutput=False)
    mask_d = nc.declare_dram_parameter("mask", [128, 256], F16, isOutput=False)
    y_d = nc.declare_dram_parameter("y", [S, D], F16, isOutput=True)

    with TileContext(nc) as tc:
        with (
            tc.tile_pool(name="static", bufs=1) as st,
            tc.tile_pool(name="ppool", bufs=6) as ppool,
            tc.tile_pool(name="rpool", bufs=3) as rpool,
            tc.tile_pool(name="ystage", bufs=3) as ystage,
            tc.tile_pool(name="psA", bufs=2, space="PSUM") as psA,
            tc.tile_pool(name="psS", bufs=2, space="PSUM") as psS,
            tc.tile_pool(name="psV", bufs=1, space="PSUM") as psV,
        ):
            mask = st.tile([128, 256], F16, name="mask", tag="mask")
            wq = st.tile([128, 2048], F16, name="wq", tag="wq")
            wk = st.tile([128, 2048], F16, name="wk", tag="wk")
            wv = st.tile([128, 2048], F16, name="wv", tag="wv")
            wo = st.tile([128, 2048], F16, name="wo", tag="wo")
            xTbig = st.tile([128, 8 * S], F16, name="xTbig", tag="xTbig")
            junk = st.tile([128, 640], F16, name="junk", tag="junk")
            junko = st.tile([128, 8], F16, name="junko", tag="junko")
            # vabig: per k-tile i, head h: cols 512i+128h..+64 = V_h, +64.. = 1
            vabig = st.tile([128, 512 * NKT], F16, name="vabig", tag="vabig")

            def xTc(k, c0, c1):
                return xTbig[:, S * k + c0 : S * k + c1]

            # ---- PE warm-up (HAM un-throttles after ~3.4us of sustained PE
            # activity; keep it busy through the whole input-DMA window so the
            # first projection starts at 2.4 GHz) + act-table preload.
            nc.vector.memset(junk[:], 0.0)
            for w in range(8):
                jacc = psA.tile([128, 512], F32, name="jacc", tag="acc")
                nc.tensor.matmul(
                    jacc[:],
                    junk[:, 0:128],
                    junk[:, 128:640],
                    start=True,
                    stop=True,
                )
            nc.scalar.activation(
                junko[:], junk[:, 0:8], mybir.ActivationFunctionType.Exp, scale=1.0
            )

            # ---- input DMAs. Every dma_start's descriptors stripe over all
            # 16 DMA engines, so one big multi-dim transfer moves at full
            # bandwidth; merging transfers cuts DGE dispatch time AND the
            # per-semaphore teardown cost in the program epilogue.
            # (3+-dim merged transfers fall off the HWDGE fast path into slow
            # software descriptor generation — keep everything plain 2D.)
            for k in range(8):
                nc.gpsimd.dma_start(
                    out=xTc(k, 0, SC), in_=xT_d[128 * k : 128 * k + 128, 0:SC]
                )
            for k in range(8):
                nc.sync.dma_start(
                    out=wq[:, 256 * k : 256 * k + 256],
                    in_=wqT_d[128 * k : 128 * k + 128, :],
                )
            for k in range(8):
                nc.scalar.dma_start(
                    out=wk[:, 256 * k : 256 * k + 256],
                    in_=wkT_d[128 * k : 128 * k + 128, :],
                )
            nc.vector.memset(
                vabig.rearrange("p (i h c) -> p i h c", i=NKT, c=128)[:, :, :, 64:128],
                1.0,
            )
            # input DMA is bandwidth-bound (~6.5MB at ~350GB/s ≈ 19us), so
            # arrival ORDER is what matters: wv then x chunk 1 before x
            # chunks 2-3, else proj(1) — and with it attention(1)'s whole
            # exp stream — waits ~20us on the tail of one big transfer.
            for k in range(8):
                nc.gpsimd.dma_start(
                    out=wv[:, 256 * k : 256 * k + 256],
                    in_=wvT_d[128 * k : 128 * k + 128, :],
                )
            for k in range(4):
                nc.sync.dma_start(
                    out=xTc(k, SC, 2 * SC),
                    in_=xT_d[128 * k : 128 * k + 128, SC : 2 * SC],
                )
            for k in range(4, 8):
                nc.gpsimd.dma_start(
                    out=xTc(k, SC, 2 * SC),
                    in_=xT_d[128 * k : 128 * k + 128, SC : 2 * SC],
                )
            nc.scalar.dma_start(out=mask[:], in_=mask_d[:])
            for cc in range(2):
                nc.scalar.dma_start(
                    out=wo[:, 1024 * cc : 1024 * cc + 1024],
                    in_=woT_d[128 * cc : 128 * cc + 128, :],
                )
            for k in range(4):
                nc.sync.dma_start(
                    out=xTc(k, 2 * SC, S), in_=xT_d[128 * k : 128 * k + 128, 2 * SC : S]
                )
            for k in range(4, 8):
                nc.gpsimd.dma_start(
                    out=xTc(k, 2 * SC, S), in_=xT_d[128 * k : 128 * k + 128, 2 * SC : S]
                )

            QT = [st.tile([128, S], F16, name=f"QT{m}", tag=f"QT{m}") for m in range(2)]
            KT = [st.tile([128, S], F16, name=f"KT{m}", tag=f"KT{m}") for m in range(2)]
            outT = [
                st.tile([128, S], F16, name=f"outT{m}", tag=f"outT{m}")
                for m in range(2)
            ]

            def proj_qk_chunk(n):
                for dst, w in ((QT, wq), (KT, wk)):
                    for m in range(2):
                        acc = psA.tile([128, SC], F32, name="acc", tag="acc")
                        for k in range(8):
                            nc.tensor.matmul(
                                acc[:],
                                w[:, 256 * k + 128 * m : 256 * k + 128 * m + 128],
                                xTc(k, SC * n, SC * n + SC),
                                start=(k == 0),
                                stop=(k == 7),
                            )
                        nc.vector.tensor_copy(dst[m][:, SC * n : SC * n + SC], acc[:])

            def proj_v(i):
                accv = psA.tile([128, 256], F32, name="accv", tag="acc")
                for k in range(8):
                    nc.tensor.matmul(
                        accv[:],
                        xTc(k, 128 * i, 128 * i + 128),
                        wv[:, 256 * k : 256 * k + 256],
                        start=(k == 0),
                        stop=(k == 7),
                    )
                # one strided evacuation into the V columns of vabig
                dst = vabig.rearrange("p (i h c) -> p i h c", i=NKT, c=128)[
                    :, i, :, 0:64
                ]
                src = accv.rearrange("p (h c) -> p h c", c=64)
                nc.vector.tensor_copy(dst, src)

            def score_exp(jq, hp, kt):
                t = kt - 4 * jq  # >=0: diagonal k-tile index
                q0 = 128 * t if t > 0 else 0  # first causal q col
                sp = psS.tile([128, 1024], F32, name="sp", tag="sp")
                spv = sp.rearrange("p (u q) -> p u q", q=SC)
                for u, base in ((0, 0), (1, 64)):
                    nc.tensor.matmul(
                        spv[:, u, q0:SC],
                        KT[hp][base : base + 64, 128 * kt : 128 * kt + 128],
                        QT[hp][base : base + 64, SC * jq + q0 : SC * jq + SC],
                        start=True,
                        stop=True,
                    )
                pt = ppool.tile([128, 1024], F16, name="pt", tag="pt")
                ptv = pt.rearrange("p (u q) -> p u q", q=SC)
                nc.scalar.activation(
                    ptv[:, :, q0:SC],
                    spv[:, :, q0:SC],
                    mybir.ActivationFunctionType.Exp,
                    scale=float(ATTN_SCALE),
                )
                if t >= 0:  # diagonal k-tile: tri-mask its 128 q cols
                    nc.vector.tensor_mul(
                        ptv[:, :, q0 : q0 + 128],
                        ptv[:, :, q0 : q0 + 128],
                        mask.rearrange("p (u q) -> p u q", q=128),
                    )
                return pt, ptv, q0

            def attention(jq, pre0=None):
                nkt = 4 * jq + 4  # causal: k-tiles 0 .. 4*jq+3
                pre = None  # pre-emitted (pt, ptv, q0) for hp1's kt=0
                for hp in range(2):
                    av = [
                        psV.tile([128, SC], F32, name=f"av{u}", tag=f"av{u}")
                        for u in range(2)
                    ]
                    last_pt = None
                    if jq == 0 and hp == 0:
                        # chunk 0 opener: the score/exp stream needs only
                        # Q/K, so emit all four k-tiles' scores+exps BEFORE
                        # the V projections (whose wv input is still in
                        # flight); the AV matmuls catch up afterwards.
                        ses = [score_exp(jq, 0, kt) for kt in range(nkt)]
                        for i in range(4):
                            proj_v(i)
                        for kt in range(nkt):
                            pt, ptv, q0 = ses[kt]
                            last_pt = pt
                            for u in range(2):
                                nc.tensor.matmul(
                                    av[u][:, q0:SC],
                                    vabig[
                                        :, 512 * kt + 128 * u : 512 * kt + 128 * u + 128
                                    ],
                                    ptv[:, u, q0:SC],
                                    start=(kt == 0),
                                    stop=(kt == nkt - 1),
                                )
                    else:
                        for kt in range(nkt):
                            if hp == 0 and pre0 is not None and kt < len(pre0):
                                pt, ptv, q0 = pre0[kt]
                            elif hp == 1 and kt == 0 and pre is not None:
                                pt, ptv, q0 = pre
                            else:
                                pt, ptv, q0 = score_exp(jq, hp, kt)
                            last_pt = pt
                            for u in range(2):
                                h = 2 * hp + u
                                nc.tensor.matmul(
                                    av[u][:, q0:SC],
                                    vabig[
                                        :, 512 * kt + 128 * h : 512 * kt + 128 * h + 128
                                    ],
                                    ptv[:, u, q0:SC],
                                    start=(kt == 0),
                                    stop=(kt == nkt - 1),
                                )
                    if hp == 0:
                        # pre-emit hp1's first score pair + exp so the exp
                        # stream isn't head-of-line-blocked behind fill work
                        # across the head-pair boundary (observed 3-4us
                        # ScalarE stalls otherwise).
                        pre = score_exp(jq, 1, 0)
                    # normalize: rows 64-127 of av hold the softmax denom
                    # replicated 64x. reciprocal_approx_fast mis-reads PSUM
                    # operands (verified on hw): den staged through SBUF.
                    last = jq == NQC - 1 and hp == 1
                    if last:
                        # tail: pipeline the normalization at 128-col piece
                        # granularity across ScalarE (den u0) and VectorE
                        # (everything else) so the first wo i-tile starts
                        # ~1us earlier than with whole-row staging.
                        dens, recs = [], []
                        for u in range(2):
                            den = rpool.tile([64, SC], F32, name="den", tag="den")
                            if u == 0:
                                nc.scalar.activation(
                                    den[:], av[u][64:128, :],
                                    mybir.ActivationFunctionType.Copy, scale=1.0,
                                )
                            else:
                                nc.vector.tensor_copy(den[:], av[u][64:128, :])
                            rec = rpool.tile([64, SC], F32, name="rec", tag="rec")
                            dens.append(den)
                            recs.append(rec)
                        # keep the PE clock warm across the normalization gap.
                        # rhs reads the final P tile so these only become
                        # ready once the tail begins (else the scheduler
                        # hoists them into earlier idle slots).
                        for w in range(6):
                            jacc = psA.tile([128, 512], F32, name="jacc", tag="acc")
                            nc.tensor.matmul(
                                jacc[:],
                                junk[:, 0:128],
                                last_pt[:, 256:768],
                                start=True,
                                stop=True,
                            )
                        for piece in range(4):
                            cl = slice(128 * piece, 128 * piece + 128)
                            gl = slice(
                                SC * jq + 128 * piece, SC * jq + 128 * piece + 128
                            )
                            for u in range(2):
                                nc.vector.reciprocal_approx_fast(
                                    recs[u][:, cl], dens[u][:, cl]
                                )
                            for u in range(2):
                                nc.vector.tensor_mul(
                                    outT[hp][64 * u : 64 * u + 64, gl],
                                    av[u][0:64, cl],
                                    recs[u][:, cl],
                                )
                    else:
                        for u in range(2):
                            den = rpool.tile([64, SC], F32, name="den", tag="den")
                            nc.vector.tensor_copy(den[:], av[u][64:128, :])
                            rec = rpool.tile([64, SC], F32, name="rec", tag="rec")
                            nc.vector.reciprocal_approx_fast(rec[:], den[:])
                            nc.vector.tensor_mul(
                                outT[hp][64 * u : 64 * u + 64, SC * jq : SC * jq + SC],
                                av[u][0:64, :],
                                rec[:],
                            )

            def wo_chunk(jq):
                last = jq == NQC - 1
                for i in range(4 * jq, 4 * jq + 4):
                    ys = ystage.tile([128, 1024], F16, name="ys", tag="ys")
                    for n in range(2):
                        if last and i >= 4 * jq + 2:
                            # attention is over; borrow the (now idle) score
                            # psum banks to double the yp slots in flight.
                            yp = psS.tile([128, 1024], F32, name="ypl", tag="sp")[
                                :, 0:512
                            ]
                        else:
                            yp = psA.tile([128, 512], F32, name="yp", tag="acc")
                        for cc in range(2):
                            nc.tensor.matmul(
                                yp[:],
                                outT[cc][:, 128 * i : 128 * i + 128],
                                wo[:, 1024 * cc + 512 * n : 1024 * cc + 512 * n + 512],
                                start=(cc == 0),
                                stop=(cc == 1),
                            )
                        ysl = ys[:, 512 * n : 512 * n + 512]
                        if last and n == 1:
                            # ACT is idle after the final exp; split the two
                            # evacuations across ScalarE and VectorE.
                            nc.scalar.activation(
                                ysl, yp[:], mybir.ActivationFunctionType.Copy,
                                scale=1.0,
                            )
                        else:
                            nc.vector.tensor_copy(ysl, yp[:])
                    eng = nc.sync if i % 2 == 0 else nc.gpsimd
                    eng.dma_start(out=y_d[128 * i : 128 * i + 128, :], in_=ys[:])

            proj_qk_chunk(0)
            pre_next = None
            for n in range(NQC):
                attention(n, pre_next)
                pre_next = None
                if n + 1 < NQC:
                    proj_qk_chunk(n + 1)
                    # pre-emit the next chunk's first 4 score/exp pairs
                    # (they read only old K columns + the just-projected Q)
                    # ahead of proj_v and wo so the exp stream restarts
                    # immediately at the chunk boundary.
                    pre_next = [score_exp(n + 1, 0, kt) for kt in range(4)]
                    for i in range(4 * (n + 1), 4 * (n + 1) + 4):
                        proj_v(i)
                wo_chunk(n)

    nc.compile()
    return nc


def _mask_np():
    # [128, 256]: tri pattern duplicated for the two row-packed heads.
    r = np.arange(128)[:, None]
    c = np.arange(128)[None, :]
    tri = (r <= c).astype(np.float16)
    return np.concatenate([tri, tri], axis=1)


def kernel(x, Wq, Wk, Wv, Wo):
    x = np.asarray(x, dtype=np.float32)
    Wq = np.asarray(Wq, dtype=np.float32)
    Wk = np.asarray(Wk, dtype=np.float32)
    Wv = np.asarray(Wv, dtype=np.float32)
    Wo = np.asarray(Wo, dtype=np.float32)

    if "nc" not in _CACHE:
        _CACHE["nc"] = _build()
    nc = _CACHE["nc"]

    masks = _mask_np()
    xT = [np.ascontiguousarray(x[b].T).astype(np.float16) for b in range(B)]
    in_maps = []
    for c in range(NCORES):
        b, g = c // 4, c % 4
        rows = slice(256 * g, 256 * g + 256)
        in_maps.append(
            {
                "xT": xT[b],
                "wqT": np.ascontiguousarray(Wq[rows].T).astype(np.float16),
                "wkT": np.ascontiguousarray(Wk[rows].T).astype(np.float16),
                "wvT": np.ascontiguousarray(Wv[rows].T).astype(np.float16),
                "woT": np.ascontiguousarray(Wo[:, rows].T).astype(np.float16),
                "mask": masks,
            }
        )

    trace = False
    trace_tmpdir = None
    if os.environ.get("KERNEL_TRACE") == "1":
        try:
            from trn_agent_boot.trn_boot import _ntff_profile_via_ctypes

            try:
                from antenv.axon_hooks import (
                    get_axon_ntff_profile_hook,
                    set_axon_ntff_profile_hook,
                )
            except ImportError:
                # This image's antenv lacks the hook-registry module; provide
                # an equivalent in-process registry so bass_utils can find it.
                import types

                import antenv

                _mod = types.ModuleType("antenv.axon_hooks")
                _mod._hook = None

                def get_axon_ntff_profile_hook(_m=_mod):
                    return _m._hook

                def set_axon_ntff_profile_hook(h, _m=_mod):
                    _m._hook = h

                _mod.get_axon_ntff_profile_hook = get_axon_ntff_profile_hook
                _mod.set_axon_ntff_profile_hook = set_axon_ntff_profile_hook
                sys.modules["antenv.axon_hooks"] = _mod
                antenv.axon_hooks = _mod

            if get_axon_ntff_profile_hook() is None:
                set_axon_ntff_profile_hook(
                    _ntff_profile_via_ctypes("/opt/axon/libaxon_pjrt.so")
                )
            trace = get_axon_ntff_profile_hook() is not None
            if trace:
                import shutil

                trace_tmpdir = "/tmp/ktrace"
                shutil.rmtree(trace_tmpdir, ignore_errors=True)
                os.makedirs(trace_tmpdir, exist_ok=True)
        except Exception:
            trace = False
            trace_tmpdir = None

    res = run_bass_kernel_spmd(
        nc, in_maps, core_ids=list(range(NCORES)), trace=trace, tmpdir=trace_tmpdir
    )
    _CACHE["exec_time_ns"] = res.exec_time_ns
    _CACHE["res"] = res
    y = np.zeros((B, S, D), dtype=np.float32)
    for c in range(NCORES):
        y[c // 4] += res.results[c]["y"].astype(np.float32)
    return y


# revision 62
# speedup vs baseline: 1.0532x; 1.0030x over previous
"""Multi-head self-attention (B=2, S=2048, D=1024, H=16, causal) on 8 NeuronCores.

Sharding: 32 (batch, head) instances -> 4 heads of one batch per core
(cores 0-3: batch 0, cores 4-7: batch 1; core c owns heads 4*(c%4) .. +3).
Wq/Wk/Wv are split by rows (head dims), Wo by columns; each core computes a
partial y[b] = attn_out_heads @ Wo_cols.T and the host sums the 4 partials
per batch at gather time (tensor-parallel reduce).

Per-core kernel. All matmuls fp16 x fp16 -> fp32 psum. No on-device
transposes:
  QT[256,2048] = wqT.T @ xT        (head-pair tiles: rows 0-63 / 64-127)
  KT likewise; V[2048,256] natural (lhsT = xT chunks), stored per k-tile in
  vabig: per head [V_h (64 cols) | ones (64 cols)].
  Scores computed transposed, blockwise [k-tile 128, q-chunk 512]:
      S^T = KT_h.T @ QT_h   -- two heads row-packed (contraction d=64 at
      partition bases 0 / 64 -> disjoint PE row groups, HW-concurrent).
      Diagonal k-tiles restrict N to the causal range (q >= key base);
      one Exp (scale=1/8) per tile on ScalarE -> P fp16 in SBUF; only the
      [128,128] diagonal block needs the 0/1 tri-mask multiply (DVE).
  AV: lhsT = [V_h | ones64] fp16 [k,128], rhs = P [k,<=512] -> psum
      [128,512] accumulated over k-tiles = unnormalized out^T (rows 0-63)
      + softmax denominators replicated over rows 64-127. Normalize via
      reciprocal_approx_fast(den) + one DVE multiply -> outT fp16.
  y = out_headsT.T @ woT -> [2048, 1024] fp16 partial, DMA'd out in
      [128,1024] blocks (host accumulates partials in fp32).

Startup: junk matmuls warm the PE HAM clock-gate during the input-DMA
window and a dummy exp pulls the ScalarE act-table load off the critical
path. Input DMAs are merged into a handful of multi-dim transfers (each
stripes across all 16 DMA engines) to minimize DGE dispatch time and the
per-semaphore program-teardown cost. The last chunk bridges its
normalization gap with junk matmuls so the PE clock never re-throttles,
runs its den staging + y evacuations on the (then idle) ScalarE, and
normalizes in 128-col pieces so each Wo tile starts as early as possible.
"""
import os
import sys

sys.path.insert(0, "/opt/trn_rl_repo")

import numpy as np

import concourse.bass as bass  # noqa: F401
import concourse.mybir as mybir
from concourse import bacc
from concourse.tile import TileContext
from concourse.bass_utils import run_bass_kernel_spmd

B, S, D = 2, 2048, 1024
H, HD = 16, 64
NCORES = 8
HPC = 4            # heads per core
SC = 512           # q-chunk width
NQC = S // SC      # 4 q-chunks
NKT = S // 128     # 16 k-tiles
F16 = mybir.dt.float16
F32 = mybir.dt.float32
ATTN_SCALE = 1.0 / np.sqrt(HD)

_CACHE = {}


def _build():
    nc = bacc.Bacc("TRN2", target_bir_lowering=False, debug=False, num_devices=NCORES)

    xT_d = nc.declare_dram_parameter("xT", [D, S], F16, isOutput=False)
    wqT_d = nc.declare_dram_parameter("wqT", [D, 256], F16, isOutput=False)
    wkT_d = nc.declare_dram_parameter("wkT", [D, 256], F16, isOutput=False)
    wvT_d = nc.declare_dram_parameter("wvT", [D, 256], F16, isOutput=False)
    woT_d = nc.declare_dram_parameter("woT", [256, D], F16, isOutput=False)
    mask_d = nc.declare_dram_parameter("mask", [128, 256], F16, isOutput=False)
    y_d = nc.declare_dram_parameter("y", [S, D], F16, isOutput=True)

    with TileContext(nc) as tc:
        with (
            tc.tile_pool(name="static", bufs=1) as st,
            tc.tile_pool(name="ppool", bufs=6) as ppool,
            tc.tile_pool(name="rpool", bufs=3) as rpool,
            tc.tile_pool(name="ystage", bufs=3) as ystage,
            tc.tile_pool(name="psA", bufs=2, space="PSUM") as psA,
            tc.tile_pool(name="psS", bufs=2, space="PSUM") as psS,
            tc.tile_pool(name="psV", bufs=1, space="PSUM") as psV,
        ):
            mask = st.tile([128, 256], F16, name="mask", tag="mask")
            wq = st.tile([128, 2048], F16, name="wq", tag="wq")
            wk = st.tile([128, 2048], F16, name="wk", tag="wk")
            wv = st.tile([128, 2048], F16, name="wv", tag="wv")
            wo = st.tile([128, 2048], F16, name="wo", tag="wo")
            xTbig = st.tile([128, 8 * S], F16, name="xTbig", tag="xTbig")
            junk = st.tile([128, 640], F16, name="junk", tag="junk")
            junko = st.tile([128, 8], F16, name="junko", tag="junko")
            # vabig: per k-tile i, head h: cols 512i+128h..+64 = V_h, +64.. = 1
            vabig = st.tile([128, 512 * NKT], F16, name="vabig", tag="vabig")

            def xTc(k, c0, c1):
                return xTbig[:, S * k + c0 : S * k + c1]

            # ---- PE warm-up (HAM un-throttles after ~3.4us of sustained PE
            # activity; keep it busy through the whole input-DMA window so the
            # first projection starts at 2.4 GHz) + act-table preload.
            nc.vector.memset(junk[:], 0.0)
            for w in range(8):
                jacc = psA.tile([128, 512], F32, name="jacc", tag="acc")
                nc.tensor.matmul(
                    jacc[:],
                    junk[:, 0:128],
                    junk[:, 128:640],
                    start=True,
                    stop=True,
                )
            nc.scalar.activation(
                junko[:], junk[:, 0:8], mybir.ActivationFunctionType.Exp, scale=1.0
            )

            # ---- input DMAs. Every dma_start's descriptors stripe over all
            # 16 DMA engines, so one big multi-dim transfer moves at full
            # bandwidth; merging transfers cuts DGE dispatch time AND the
            # per-semaphore teardown cost in the program epilogue.
            # (3+-dim merged transfers fall off the HWDGE fast path into slow
            # software descriptor generation — keep everything plain 2D.)
            for k in range(8):
                nc.gpsimd.dma_start(
                    out=xTc(k, 0, SC), in_=xT_d[128 * k : 128 * k + 128, 0:SC]
                )
            for k in range(8):
                nc.sync.dma_start(
                    out=wq[:, 256 * k : 256 * k + 256],
                    in_=wqT_d[128 * k : 128 * k + 128, :],
                )
            for k in range(8):
                nc.scalar.dma_start(
                    out=wk[:, 256 * k : 256 * k + 256],
                    in_=wkT_d[128 * k : 128 * k + 128, :],
                )
            nc.vector.memset(
                vabig.rearrange("p (i h c) -> p i h c", i=NKT, c=128)[:, :, :, 64:128],
                1.0,
            )
            # input DMA is bandwidth-bound (~6.5MB at ~350GB/s ≈ 19us), so
            # arrival ORDER is what matters: wv then x chunk 1 before x
            # chunks 2-3, else proj(1) — and with it attention(1)'s whole
            # exp stream — waits ~20us on the tail of one big transfer.
            for k in range(8):
                nc.gpsimd.dma_start(
                    out=wv[:, 256 * k : 256 * k + 256],
                    in_=wvT_d[128 * k : 128 * k + 128, :],
                )
            for k in range(4):
                nc.sync.dma_start(
                    out=xTc(k, SC, 2 * SC),
                    in_=xT_d[128 * k : 128 * k + 128, SC : 2 * SC],
                )
            for k in range(4, 8):
                nc.gpsimd.dma_start(
                    out=xTc(k, SC, 2 * SC),
                    in_=xT_d[128 * k : 128 * k + 128, SC : 2 * SC],
                )
            nc.scalar.dma_start(out=mask[:], in_=mask_d[:])
            for cc in range(2):
                nc.scalar.dma_start(
                    out=wo[:, 1024 * cc : 1024 * cc + 1024],
                    in_=woT_d[128 * cc : 128 * cc + 128, :],
                )
            for k in range(4):
                nc.sync.dma_start(
                    out=xTc(k, 2 * SC, S), in_=xT_d[128 * k : 128 * k + 128, 2 * SC : S]
                )
            for k in range(4, 8):
                nc.gpsimd.dma_start(
                    out=xTc(k, 2 * SC, S), in_=xT_d[128 * k : 128 * k + 128, 2 * SC : S]
                )

            QT = [st.tile([128, S], F16, name=f"QT{m}", tag=f"QT{m}") for m in range(2)]
            KT = [st.tile([128, S], F16, name=f"KT{m}", tag=f"KT{m}") for m in range(2)]
            outT = [
                st.tile([128, S], F16, name=f"outT{m}", tag=f"outT{m}")
                for m in range(2)
            ]

            def proj_qk_chunk(n):
                for dst, w in ((QT, wq), (KT, wk)):
                    for m in range(2):
                        acc = psA.tile([128, SC], F32, name="acc", tag="acc")
                        for k in range(8):
                            nc.tensor.matmul(
                                acc[:],
                                w[:, 256 * k + 128 * m : 256 * k + 128 * m + 128],
                                xTc(k, SC * n, SC * n + SC),
                                start=(k == 0),
                                stop=(k == 7),
                            )
                        nc.vector.tensor_copy(dst[m][:, SC * n : SC * n + SC], acc[:])

            def proj_v(i):
                accv = psA.tile([128, 256], F32, name="accv", tag="acc")
                for k in range(8):
                    nc.tensor.matmul(
                        accv[:],
                        xTc(k, 128 * i, 128 * i + 128),
                        wv[:, 256 * k : 256 * k + 256],
                        start=(k == 0),
                        stop=(k == 7),
                    )
                # one strided evacuation into the V columns of vabig
                dst = vabig.rearrange("p (i h c) -> p i h c", i=NKT, c=128)[
                    :, i, :, 0:64
                ]
                src = accv.rearrange("p (h c) -> p h c", c=64)
                nc.vector.tensor_copy(dst, src)

            def score_exp(jq, hp, kt):
                t = kt - 4 * jq  # >=0: diagonal k-tile index
                q0 = 128 * t if t > 0 else 0  # first causal q col
                sp = psS.tile([128, 1024], F32, name="sp", tag="sp")
                spv = sp.rearrange("p (u q) -> p u q", q=SC)
                for u, base in ((0, 0), (1, 64)):
                    nc.tensor.matmul(
                        spv[:, u, q0:SC],
                        KT[hp][base : base + 64, 128 * kt : 128 * kt + 128],
                        QT[hp][base : base + 64, SC * jq + q0 : SC * jq + SC],
                        start=True,
                        stop=True,
                    )
                pt = ppool.tile([128, 1024], F16, name="pt", tag="pt")
                ptv = pt.rearrange("p (u q) -> p u q", q=SC)
                nc.scalar.activation(
                    ptv[:, :, q0:SC],
                    spv[:, :, q0:SC],
                    mybir.ActivationFunctionType.Exp,
                    scale=float(ATTN_SCALE),
                )
                if t >= 0:  # diagonal k-tile: tri-mask its 128 q cols
                    nc.vector.tensor_mul(
                        ptv[:, :, q0 : q0 + 128],
                        ptv[:, :, q0 : q0 + 128],
                        mask.rearrange("p (u q) -> p u q", q=128),
                    )
                return pt, ptv, q0

            def attention(jq, pre0=None):
                nkt = 4 * jq + 4  # causal: k-tiles 0 .. 4*jq+3
                pre = None  # pre-emitted (pt, ptv, q0) for hp1's kt=0
                for hp in range(2):
                    av = [
                        psV.tile([128, SC], F32, name=f"av{u}", tag=f"av{u}")
                        for u in range(2)
                    ]
                    last_pt = None
                    if jq == 0 and hp == 0:
                        # chunk 0 opener: the score/exp stream needs only
                        # Q/K, so emit all four k-tiles' scores+exps BEFORE
                        # the V projections (whose wv input is still in
                        # flight); the AV matmuls catch up afterwards.
                        ses = [score_exp(jq, 0, kt) for kt in range(nkt)]
                        for i in range(4):
                            proj_v(i)
                        for kt in range(nkt):
                            pt, ptv, q0 = ses[kt]
                            last_pt = pt
                            for u in range(2):
                                nc.tensor.matmul(
                                    av[u][:, q0:SC],
                                    vabig[
                                        :, 512 * kt + 128 * u : 512 * kt + 128 * u + 128
                                    ],
                                    ptv[:, u, q0:SC],
                                    start=(kt == 0),
                                    stop=(kt == nkt - 1),
                                )
                    else:
                        for kt in range(nkt):
                            if hp == 0 and pre0 is not None and kt < len(pre0):
                                pt, ptv, q0 = pre0[kt]
                            elif hp == 1 and kt == 0 and pre is not None:
                                pt, ptv, q0 = pre
                            else:
                                pt, ptv, q0 = score_exp(jq, hp, kt)
                            last_pt = pt
                            for u in range(2):
                                h = 2 * hp + u
                                nc.tensor.matmul(
                                    av[u][:, q0:SC],
                                    vabig[
                                        :, 512 * kt + 128 * h : 512 * kt + 128 * h + 128
                                    ],
                                    ptv[:, u, q0:SC],
                                    start=(kt == 0),
                                    stop=(kt == nkt - 1),
                                )
                    if hp == 0:
                        # pre-emit hp1's first score pair + exp so the exp
                        # stream isn't head-of-line-blocked behind fill work
                        # across the head-pair boundary (observed 3-4us
                        # ScalarE stalls otherwise).
                        pre = score_exp(jq, 1, 0)
                    # normalize: rows 64-127 of av hold the softmax denom
                    # replicated 64x. reciprocal_approx_fast mis-reads PSUM
                    # operands (verified on hw): den staged through SBUF.
                    last = jq == NQC - 1 and hp == 1
                    if last:
                        # tail: pipeline the normalization at 128-col piece
                        # granularity across ScalarE (den u0) and VectorE
                        # (everything else) so the first wo i-tile starts
                        # ~1us earlier than with whole-row staging.
                        dens, recs = [], []
                        for u in range(2):
                            den = rpool.tile([64, SC], F32, name="den", tag="den")
                            if u == 0:
                                nc.scalar.activation(
                                    den[:], av[u][64:128, :],
                                    mybir.ActivationFunctionType.Copy, scale=1.0,
                                )
                            else:
                                nc.vector.tensor_copy(den[:], av[u][64:128, :])
                            rec = rpool.tile([64, SC], F32, name="rec", tag="rec")
                            dens.append(den)
                            recs.append(rec)
                        # keep the PE clock warm across the normalization gap.
                        # rhs reads the final P tile so these only become
                        # ready once the tail begins (else the scheduler
                        # hoists them into earlier idle slots).
                        for w in range(6):
                            jacc = psA.tile([128, 512], F32, name="jacc", tag="acc")
                            nc.tensor.matmul(
                                jacc[:],
                                junk[:, 0:128],
                                last_pt[:, 256:768],
                                start=True,
                                stop=True,
                            )
                        for piece in range(4):
                            cl = slice(128 * piece, 128 * piece + 128)
                            gl = slice(
                                SC * jq + 128 * piece, SC * jq + 128 * piece + 128
                            )
                            for u in range(2):
                                nc.vector.reciprocal_approx_fast(
                                    recs[u][:, cl], dens[u][:, cl]
                                )
                            for u in range(2):
                                nc.vector.tensor_mul(
                                    outT[hp][64 * u : 64 * u + 64, gl],
                                    av[u][0:64, cl],
                                    recs[u][:, cl],
                                )
                    else:
                        for u in range(2):
                            den = rpool.tile([64, SC], F32, name="den", tag="den")
                            nc.vector.tensor_copy(den[:], av[u][64:128, :])
                            rec = rpool.tile([64, SC], F32, name="rec", tag="rec")
                            nc.vector.reciprocal_approx_fast(rec[:], den[:])
                            nc.vector.tensor_mul(
                                outT[hp][64 * u : 64 * u + 64, SC * jq : SC * jq + SC],
                                av[u][0:64, :],
                                rec[:],
                            )

            def wo_chunk(jq):
                last = jq == NQC - 1
                for i in range(4 * jq, 4 * jq + 4):
                    ys = ystage.tile([128, 1024], F16, name="ys", tag="ys")
                    for n in range(2):
                        if last and i >= 4 * jq + 2:
                            # attention is over; borrow the (now idle) score
                            # psum banks to double the yp slots in flight.
                            yp = psS.tile([128, 1024], F32, name="ypl", tag="sp")[
                                :, 0:512
                            ]
                        else:
                            yp = psA.tile([128, 512], F32, name="yp", tag="acc")
                        for cc in range(2):
                            nc.tensor.matmul(
                                yp[:],
                                outT[cc][:, 128 * i : 128 * i + 128],
                                wo[:, 1024 * cc + 512 * n : 1024 * cc + 512 * n + 512],
                                start=(cc == 0),
                                stop=(cc == 1),
                            )
                        ysl = ys[:, 512 * n : 512 * n + 512]
                        if last and n == 1:
                            # ACT is idle after the final exp; split the two
                            # evacuations across ScalarE and VectorE.
                            nc.scalar.activation(
                                ysl, yp[:], mybir.ActivationFunctionType.Copy,
                                scale=1.0,
                            )
                        else:
                            nc.vector.tensor_copy(ysl, yp[:])
                    eng = nc.sync if i % 2 == 0 else nc.gpsimd
                    eng.dma_start(out=y_d[128 * i : 128 * i + 128, :], in_=ys[:])

            proj_qk_chunk(0)
            pre_next = None
            for n in range(NQC):
                attention(n, pre_next)
                pre_next = None
                if n + 1 < NQC:
                    proj_qk_chunk(n + 1)
                    # pre-emit the next chunk's first 4 score/exp pairs
                    # (they read only old K columns + the just-projected Q)
                    # ahead of proj_v and wo so the exp stream restarts
                    # immediately at the chunk boundary.
                    pre_next = [score_exp(n + 1, 0, kt) for kt in range(4 if n == 0 else 8)]
                    for i in range(4 * (n + 1), 4 * (n + 1) + 4):
                        proj_v(i)
                wo_chunk(n)

    nc.compile()
    return nc


def _mask_np():
    # [128, 256]: tri pattern duplicated for the two row-packed heads.
    r = np.arange(128)[:, None]
    c = np.arange(128)[None, :]
    tri = (r <= c).astype(np.float16)
    return np.concatenate([tri, tri], axis=1)


def kernel(x, Wq, Wk, Wv, Wo):
    x = np.asarray(x, dtype=np.float32)
    Wq = np.asarray(Wq, dtype=np.float32)
    Wk = np.asarray(Wk, dtype=np.float32)
    Wv = np.asarray(Wv, dtype=np.float32)
    Wo = np.asarray(Wo, dtype=np.float32)

    if "nc" not in _CACHE:
        _CACHE["nc"] = _build()
    nc = _CACHE["nc"]

    masks = _mask_np()
    xT = [np.ascontiguousarray(x[b].T).astype(np.float16) for b in range(B)]
    in_maps = []
    for c in range(NCORES):
        b, g = c // 4, c % 4
        rows = slice(256 * g, 256 * g + 256)
        in_maps.append(
            {
                "xT": xT[b],
                "wqT": np.ascontiguousarray(Wq[rows].T).astype(np.float16),
                "wkT": np.ascontiguousarray(Wk[rows].T).astype(np.float16),
                "wvT": np.ascontiguousarray(Wv[rows].T).astype(np.float16),
                "woT": np.ascontiguousarray(Wo[:, rows].T).astype(np.float16),
                "mask": masks,
            }
        )

    trace = False
    trace_tmpdir = None
    if os.environ.get("KERNEL_TRACE") == "1":
        try:
            from trn_agent_boot.trn_boot import _ntff_profile_via_ctypes

            try:
                from antenv.axon_hooks import (
                    get_axon_ntff_profile_hook,
                    set_axon_ntff_profile_hook,
                )
            except ImportError:
                # This image's antenv lacks the hook-registry module; provide
                # an equivalent in-process registry so bass_utils can find it.
                import types

                import antenv

                _mod = types.ModuleType("antenv.axon_hooks")
                _mod._hook = None

                def get_axon_ntff_profile_hook(_m=_mod):
                    return _m._hook

                def set_axon_ntff_profile_hook(h, _m=_mod):
                    _m._hook = h

                _mod.get_axon_ntff_profile_hook = get_axon_ntff_profile_hook
                _mod.set_axon_ntff_profile_hook = set_axon_ntff_profile_hook
                sys.modules["antenv.axon_hooks"] = _mod
                antenv.axon_hooks = _mod

            if get_axon_ntff_profile_hook() is None:
                set_axon_ntff_profile_hook(
                    _ntff_profile_via_ctypes("/opt/axon/libaxon_pjrt.so")
                )
            trace = get_axon_ntff_profile_hook() is not None
            if trace:
                import shutil

                trace_tmpdir = "/tmp/ktrace"
                shutil.rmtree(trace_tmpdir, ignore_errors=True)
                os.makedirs(trace_tmpdir, exist_ok=True)
        except Exception:
            trace = False
            trace_tmpdir = None

    res = run_bass_kernel_spmd(
        nc, in_maps, core_ids=list(range(NCORES)), trace=trace, tmpdir=trace_tmpdir
    )
    _CACHE["exec_time_ns"] = res.exec_time_ns
    _CACHE["res"] = res
    y = np.zeros((B, S, D), dtype=np.float32)
    for c in range(NCORES):
        y[c // 4] += res.results[c]["y"].astype(np.float32)
    return y
